# revision 1
# baseline (speedup 1.0000x reference)
"""GroupedExperts (MoE bmm path) forward on 8 Trainium2 NeuronCores.

Reference (per expert e):
    h   = silu(x[e] @ w1[e]) * (x[e] @ w3[e])
    out = h @ w2[e]
with E=8, T=4096, D=2048, H=1024, fp32 inputs.

Sharding: expert-parallel — core e owns expert e (no cross-core traffic).

Device kernel design (per core):
  Host stages inputs as bf16 with x pre-transposed to xT [D, T] so every
  matmul consumes its natural layout (no on-device transposes):
    m1/m2: aT/bT[hm, tblk] = sum_dk w1/w3[dk, hm].T @ xT[dk, tblk]
           (lhsT = weight tile [128(D) x 128(H)], rhs = xT tile [128(D) x 512(T)])
    hT    = silu(aT) * bT                     (ACT + DVE, bf16 result)
    m3:    out[tm, dn] = sum_hk hT[hk, tm].T @ w2[hk, dn]
           (lhsT = hT tile [128(H) x 128(T)], rhs = w2 tile [128(H) x 512(D)])
  PSUM accumulates in fp32; out is written fp32 in natural [T, D] layout.
  Weights stay SBUF-resident (~96KB/partition); xT / hT are double-buffered
  per 512-token block so DMA and PE overlap.
"""

import numpy as np
import ml_dtypes

import concourse.bass as bass
import concourse.mybir as mybir
import concourse.tile as tile
from concourse import bacc
from concourse.bass_utils import run_bass_kernel_spmd

E, T, D, H = 8, 4096, 2048, 1024
NCORES = 8
P = 128               # partition dim
TBLK = 512            # token block = moving free dim for m1/m2
NTBLK = T // TBLK     # 8
NDK = D // P          # 16 contraction tiles over D
NHM = H // P          # 8 tiles over H
DBLK = 512            # D chunk = moving free dim for m3
NDN = D // DBLK       # 4
NTSUB = TBLK // P     # 4

BF16 = mybir.dt.bfloat16
F32 = mybir.dt.float32

_CACHE: dict = {}


def _build_module():
    if "nc" in _CACHE:
        return _CACHE["nc"]

    nc = bacc.Bacc(
        "TRN2",
        target_bir_lowering=False,
        debug=False,
        enable_asserts=False,
        num_devices=NCORES,
    )

    xt_d = nc.dram_tensor("xt", [D, T], BF16, kind="ExternalInput").ap()
    w1_d = nc.dram_tensor("w1", [D, H], BF16, kind="ExternalInput").ap()
    w3_d = nc.dram_tensor("w3", [D, H], BF16, kind="ExternalInput").ap()
    w2_d = nc.dram_tensor("w2", [H, D], BF16, kind="ExternalInput").ap()
    out_d = nc.dram_tensor("out", [T, D], F32, kind="ExternalOutput").ap()

    with tile.TileContext(nc) as tc:
        with (
            tc.tile_pool(name="wpool", bufs=1) as wpool,
            tc.tile_pool(name="xpool", bufs=2) as xpool,
            tc.tile_pool(name="hpool", bufs=2) as hpool,
            tc.tile_pool(name="spool", bufs=4) as spool,
            tc.tile_pool(name="opool", bufs=4) as opool,
            tc.tile_pool(name="psab", bufs=2, space="PSUM") as psab,
            tc.tile_pool(name="pso", bufs=2, space="PSUM") as pso,
        ):
            # Resident weights: distinct tags -> one persistent slot each.
            w1_t = []
            w3_t = []
            for k in range(NDK):
                t1 = wpool.tile([P, H], BF16, tag=f"w1_{k}")
                t3 = wpool.tile([P, H], BF16, tag=f"w3_{k}")
                nc.sync.dma_start(t1[:], w1_d[k * P:(k + 1) * P, :])
                nc.sync.dma_start(t3[:], w3_d[k * P:(k + 1) * P, :])
                w1_t.append(t1)
                w3_t.append(t3)
            w2_t = []
            for k in range(NHM):
                t2 = wpool.tile([P, D], BF16, tag=f"w2_{k}")
                nc.sync.dma_start(t2[:], w2_d[k * P:(k + 1) * P, :])
                w2_t.append(t2)

            for i in range(NTBLK):
                ts = i * TBLK
                xt_t = []
                for k in range(NDK):
                    xt = xpool.tile([P, TBLK], BF16, tag=f"x_{k}")
                    nc.sync.dma_start(
                        xt[:], xt_d[k * P:(k + 1) * P, ts:ts + TBLK]
                    )
                    xt_t.append(xt)

                hts = []
                for hm in range(NHM):
                    hs = hm * P
                    pa = psab.tile([P, TBLK], F32, tag="pa")
                    pb = psab.tile([P, TBLK], F32, tag="pb")
                    for k in range(NDK):
                        nc.tensor.matmul(
                            pa[:], w1_t[k][:, hs:hs + P], xt_t[k][:],
                            start=(k == 0), stop=(k == NDK - 1),
                        )
                    for k in range(NDK):
                        nc.tensor.matmul(
                            pb[:], w3_t[k][:, hs:hs + P], xt_t[k][:],
                            start=(k == 0), stop=(k == NDK - 1),
                        )
                    sil = spool.tile([P, TBLK], BF16, tag="sil")
                    nc.scalar.activation(
                        sil[:], pa[:], mybir.ActivationFunctionType.Silu
                    )
                    ht = hpool.tile([P, TBLK], BF16, tag=f"h_{hm}")
                    nc.vector.tensor_mul(ht[:], sil[:], pb[:])
                    hts.append(ht)

                for tm in range(NTSUB):
                    tsub = ts + tm * P
                    for dn in range(NDN):
                        dsl = dn * DBLK
                        po = pso.tile([P, DBLK], F32, tag="po")
                        for hk in range(NHM):
                            nc.tensor.matmul(
                                po[:],
                                hts[hk][:, tm * P:(tm + 1) * P],
                                w2_t[hk][:, dsl:dsl + DBLK],
                                start=(hk == 0), stop=(hk == NHM - 1),
                            )
                        ot = opool.tile([P, DBLK], F32, tag="o")
                        nc.vector.tensor_copy(ot[:], po[:])
                        nc.sync.dma_start(
                            out_d[tsub:tsub + P, dsl:dsl + DBLK], ot[:]
                        )

    nc.compile()
    _CACHE["nc"] = nc
    return nc


def _stage_inputs(x, w1, w2, w3):
    """Per-expert bf16 staging; x pre-transposed to [D, T]."""
    bf = ml_dtypes.bfloat16
    in_maps = []
    for e in range(E):
        in_maps.append({
            "xt": np.ascontiguousarray(x[e].astype(bf).T),
            "w1": np.ascontiguousarray(w1[e].astype(bf)),
            "w3": np.ascontiguousarray(w3[e].astype(bf)),
            "w2": np.ascontiguousarray(w2[e].astype(bf)),
        })
    return in_maps


def kernel(x, w1, w2, w3):
    assert x.shape == (E, T, D) and w1.shape == (E, D, H)
    assert w2.shape == (E, H, D) and w3.shape == (E, D, H)
    nc = _build_module()
    in_maps = _stage_inputs(x, w1, w2, w3)
    res = run_bass_kernel_spmd(nc, in_maps, core_ids=list(range(NCORES)))
    out = np.stack([res.results[e]["out"] for e in range(E)], axis=0)
    return out.astype(np.float32)



# revision 2
# speedup vs baseline: 4.8568x; 4.8568x over previous
"""GroupedExperts (MoE bmm path) forward on 8 Trainium2 NeuronCores — v2.

Per expert e (one core each):
    h   = silu(x[e] @ w1[e]) * (x[e] @ w3[e]);  out = h @ w2[e]
E=8, T=4096, D=2048, H=1024, fp32 interface; bf16 on device.

v2 vs v1:
  - All inputs host-packed into [128, *] layouts so every DMA is one large
    contiguous-per-partition transfer (x: 8 x 2MB; weights: 3 x 4MB).
  - Device output is bf16 (halves out traffic + 2x DVE drain); host upcasts.
  - PSUM: pa/pb bufs=3 + po bufs=2 = 8 banks, deeper PE run-ahead.
  - Output staged per 128-token row block [128, 2048] -> 32 DMAs of 512KB.
"""

import numpy as np
import ml_dtypes

import concourse.mybir as mybir
import concourse.tile as tile
from concourse import bacc
from concourse.bass_utils import run_bass_kernel_spmd

E, T, D, H = 8, 4096, 2048, 1024
NCORES = 8
P = 128
TBLK = 512
NTBLK = T // TBLK          # 8
NDK = D // P               # 16
NHM = H // P               # 8
DBLK = 512
NDN = D // DBLK            # 4
NTSUB = TBLK // P          # 4
XCOLS = NTBLK * NDK * TBLK     # 65536
W1COLS = NDK * H               # 16384
W2COLS = NHM * D               # 16384

BF16 = mybir.dt.bfloat16
F32 = mybir.dt.float32

_CACHE: dict = {}


def _build_module():
    if "nc" in _CACHE:
        return _CACHE["nc"]

    nc = bacc.Bacc(
        "TRN2",
        target_bir_lowering=False,
        debug=False,
        enable_asserts=False,
        num_devices=NCORES,
    )

    xp_d = nc.dram_tensor("xp", [P, XCOLS], BF16, kind="ExternalInput").ap()
    w1_d = nc.dram_tensor("w1", [P, W1COLS], BF16, kind="ExternalInput").ap()
    w3_d = nc.dram_tensor("w3", [P, W1COLS], BF16, kind="ExternalInput").ap()
    w2_d = nc.dram_tensor("w2", [P, W2COLS], BF16, kind="ExternalInput").ap()
    out_d = nc.dram_tensor("out", [T, D], BF16, kind="ExternalOutput").ap()

    with tile.TileContext(nc) as tc:
        with (
            tc.tile_pool(name="wpool", bufs=1) as wpool,
            tc.tile_pool(name="xpool", bufs=2) as xpool,
            tc.tile_pool(name="hpool", bufs=2) as hpool,
            tc.tile_pool(name="spool", bufs=4) as spool,
            tc.tile_pool(name="opool", bufs=3) as opool,
            tc.tile_pool(name="psab", bufs=3, space="PSUM") as psab,
            tc.tile_pool(name="pso", bufs=2, space="PSUM") as pso,
        ):
            w1s = wpool.tile([P, W1COLS], BF16, tag="w1")
            w3s = wpool.tile([P, W1COLS], BF16, tag="w3")
            w2s = wpool.tile([P, W2COLS], BF16, tag="w2")
            nc.sync.dma_start(w1s[:], w1_d[:, :])
            nc.sync.dma_start(w3s[:], w3_d[:, :])
            nc.sync.dma_start(w2s[:], w2_d[:, :])

            for i in range(NTBLK):
                xt = xpool.tile([P, NDK * TBLK], BF16, tag="x")
                nc.sync.dma_start(
                    xt[:], xp_d[:, i * NDK * TBLK:(i + 1) * NDK * TBLK]
                )

                hts = []
                for hm in range(NHM):
                    hs = hm * P
                    pa = psab.tile([P, TBLK], F32, tag="pa")
                    pb = psab.tile([P, TBLK], F32, tag="pb")
                    for k in range(NDK):
                        nc.tensor.matmul(
                            pa[:], w1s[:, k * H + hs:k * H + hs + P],
                            xt[:, k * TBLK:(k + 1) * TBLK],
                            start=(k == 0), stop=(k == NDK - 1),
                        )
                    for k in range(NDK):
                        nc.tensor.matmul(
                            pb[:], w3s[:, k * H + hs:k * H + hs + P],
                            xt[:, k * TBLK:(k + 1) * TBLK],
                            start=(k == 0), stop=(k == NDK - 1),
                        )
                    sil = spool.tile([P, TBLK], BF16, tag="sil")
                    nc.scalar.activation(
                        sil[:], pa[:], mybir.ActivationFunctionType.Silu
                    )
                    ht = hpool.tile([P, TBLK], BF16, tag=f"h_{hm}")
                    nc.vector.tensor_mul(ht[:], sil[:], pb[:])
                    hts.append(ht)

                for tm in range(NTSUB):
                    trow = i * TBLK + tm * P
                    ot = opool.tile([P, D], BF16, tag="o")
                    for dn in range(NDN):
                        dsl = dn * DBLK
                        po = pso.tile([P, DBLK], F32, tag="po")
                        for hk in range(NHM):
                            nc.tensor.matmul(
                                po[:],
                                hts[hk][:, tm * P:(tm + 1) * P],
                                w2s[:, hk * D + dsl:hk * D + dsl + DBLK],
                                start=(hk == 0), stop=(hk == NHM - 1),
                            )
                        nc.vector.tensor_copy(ot[:, dsl:dsl + DBLK], po[:])
                    nc.sync.dma_start(out_d[trow:trow + P, :], ot[:])

    nc.compile()
    _CACHE["nc"] = nc
    return nc


def _stage_inputs(x, w1, w2, w3):
    """Per-expert bf16 packed staging (see module docstring for layouts)."""
    bf = ml_dtypes.bfloat16
    in_maps = []
    for e in range(E):
        xT = np.ascontiguousarray(x[e].astype(bf).T)      # [D, T]
        xv = xT.reshape(NDK, P, NTBLK, TBLK)
        xp = np.ascontiguousarray(
            xv.transpose(1, 2, 0, 3)).reshape(P, XCOLS)    # [p, i, k, t]
        w1p = np.ascontiguousarray(
            w1[e].astype(bf).reshape(NDK, P, H).transpose(1, 0, 2)
        ).reshape(P, W1COLS)
        w3p = np.ascontiguousarray(
            w3[e].astype(bf).reshape(NDK, P, H).transpose(1, 0, 2)
        ).reshape(P, W1COLS)
        w2p = np.ascontiguousarray(
            w2[e].astype(bf).reshape(NHM, P, D).transpose(1, 0, 2)
        ).reshape(P, W2COLS)
        in_maps.append({"xp": xp, "w1": w1p, "w3": w3p, "w2": w2p})
    return in_maps


def kernel(x, w1, w2, w3):
    assert x.shape == (E, T, D) and w1.shape == (E, D, H)
    assert w2.shape == (E, H, D) and w3.shape == (E, D, H)
    nc = _build_module()
    in_maps = _stage_inputs(x, w1, w2, w3)
    res = run_bass_kernel_spmd(nc, in_maps, core_ids=list(range(NCORES)))
    out = np.stack([res.results[e]["out"] for e in range(E)], axis=0)
    return out.astype(np.float32)
